# revision 1
# baseline (speedup 1.0000x reference)
"""Trainium2 Bass kernel for the pairwise concordance-index loss.

reference:
    loss = sum_{i<j, f_i=f_j=1} relu((p_i-p_j)(t_i-t_j)) / 100 / n_pairs

Math:
  M[i,j] = f_i f_j (p_i-p_j)(t_i-t_j) = A^T B, rank 4:
      A = [f*u, f, f*p, f*t],  B = [f, f*u, -f*t, -f*p],  u = p*t
  (flags fold in because relu(f_i f_j x) = f_i f_j relu(x) for 0/1 flags)
  sum relu(M) = 0.5*(sum M + sum |M|); sum M has an O(B) closed form done
  on the host in fp64; sum |M| is the O(B^2) part done on device.

Device decomposition (8 cores, identical program, data-sharded):
  64 row-blocks of 128 rows; core k owns blocks 8k..8k+7 as two gangs of
  4. Each block processes cyclic column-offsets e=0..32 (cols 128a+128e
  mod 8192): e=1..31 at weight 1; e=0 / e=32 at weight 0.5 via
  0.5-pre-scaled slab appendices (host-side), so all device sums have
  uniform weight.

Device structure per gang (4 row-blocks in lockstep):
  K=4 bf16 matmuls generate M. The 4 blocks' matmuls are packed into
  disjoint 32-row PE groups via tile_position (rows 0/32/64/96) and run
  CONCURRENTLY (~3x PE throughput; K=4 matmuls never warm the HAM clock,
  so concurrency is the only lever). Each "quad" (4 concurrent N<=512
  matmuls) fills the 4 banks of one [128, 4, 512] PSUM tile, which is
  consumed by ONE abs-row-sum job on either the DVE
  (tensor_reduce(apply_absolute_value, axis=XY)) or the ScalarE
  (activation(Abs, accum_out)), alternating to balance both engines.
"""

import numpy as np

B = 8192
P = 128
NCORE = 8
ABLK = 8            # row-blocks per core (2 gangs of 4)
BMAIN = 5120        # main slab: 128*(7 + 33)
BCOLS = BMAIN + 2 * ABLK * P
E0_OFF = BMAIN                 # 0.5*cols[128a ..+128) at E0_OFF+128a
E32_OFF = BMAIN + ABLK * P     # 0.5*cols[128a+4096 ..+128) at E32_OFF+128a

# per gang: Q1..Q7 (N=512 quads), Q8 (N=384 quad), Q9 (e32|e0 two N=128 quads)
NJOBS = 36          # 18 two-bank tiles per gang x 2 gangs

_cache = {}


def _build():
    """Build + compile the Bass module (once per process)."""
    import concourse.bacc as bacc
    import concourse.tile as tile
    import concourse.mybir as mybir

    f32 = mybir.dt.float32
    bf16 = mybir.dt.bfloat16
    nc = bacc.Bacc("TRN2", target_bir_lowering=False, debug=False, num_devices=NCORE)

    a_dram = nc.dram_tensor("a_rows", [P, 2 * P], bf16, kind="ExternalInput")
    b_dram = nc.dram_tensor("b_cols", [4, BCOLS], bf16, kind="ExternalInput")
    acc_dram = nc.dram_tensor("acc", [P, NJOBS], f32, kind="ExternalOutput")

    with tile.TileContext(nc) as tc:
        with (
            tc.tile_pool(name="inp", bufs=1) as inp_pool,
            tc.tile_pool(name="accp", bufs=1) as acc_pool,
            tc.tile_pool(name="ps", bufs=4, space="PSUM") as ps,
        ):
            a_sb = inp_pool.tile([P, 2 * P], bf16)
            nc.sync.dma_start(a_sb[:, :], a_dram.ap()[:, :])
            # replicate the 4 B-factor rows into all four 32-row groups;
            # chunk columns so the first-needed cols land first
            b_sb = inp_pool.tile([P, BCOLS], bf16)
            CUT = 2688
            for q in range(4):
                nc.sync.dma_start(
                    b_sb[32 * q:32 * q + 4, 0:CUT], b_dram.ap()[:, 0:CUT]
                )
            for q in range(4):
                nc.sync.dma_start(
                    b_sb[32 * q:32 * q + 4, CUT:BCOLS], b_dram.ap()[:, CUT:BCOLS]
                )

            acc_sb = acc_pool.tile([P, NJOBS], f32)

            job = 0
            for g in range(2):          # gangs: row-blocks 4g..4g+3
                def pair(poff, coff_of_a, n, engines, g=g):
                    """One quad split across two 2-bank tiles + their reduces.
                    engines: (engine for tile A [q0,q1], engine for tile B)."""
                    nonlocal job
                    tiles = (
                        ps.tile([P, 2, 512], f32, tag="q", name=f"qa{job}"),
                        ps.tile([P, 2, 512], f32, tag="q", name=f"qb{job}"),
                    )
                    for q in range(4):
                        coff = coff_of_a(4 * g + q)
                        nc.tensor.matmul(
                            tiles[q // 2][:, q % 2, poff:poff + n],
                            a_sb[32 * q:32 * q + 4, P * g:P * g + P],
                            b_sb[32 * q:32 * q + 4, coff:coff + n],
                            start=True,
                            stop=True,
                            tile_position=(32 * q, 0),
                        )
                    return tiles

                def reduce_tile(red, use_dve):
                    nonlocal job
                    if use_dve:
                        nc.vector.tensor_reduce(
                            acc_sb[:, job:job + 1], red,
                            axis=mybir.AxisListType.XY, op=mybir.AluOpType.add,
                            apply_absolute_value=True,
                        )
                    else:
                        nc.scalar.activation(
                            red, red,
                            mybir.ActivationFunctionType.Abs,
                            accum_out=acc_sb[:, job:job + 1],
                        )
                    job += 1

                # Q1..Q7: e=1..28 (N=512). tileA->ACT, tileB->DVE, except the
                # last quad of gang 1 sends both to ACT for balance.
                for s in range(7):
                    both_act = (g == 1 and s == 6)
                    tA, tB = pair(0, lambda a, s=s: P * a + 128 * (1 + 4 * s), 512,
                                  None)
                    reduce_tile(tA[:, :, :], use_dve=False)
                    reduce_tile(tB[:, :, :], use_dve=not both_act)
                # Q8: e29..31 (N=384) -> DVE
                tA, tB = pair(0, lambda a: P * a + 128 * 29, 384, None)
                reduce_tile(tA[:, :, 0:384], use_dve=True)
                reduce_tile(tB[:, :, 0:384], use_dve=True)
                # Q9: e32 then e0 (both N=128, pre-scaled) -> DVE
                t9 = None
                tA, tB = pair(0, lambda a: E32_OFF + P * a, 128, None)
                for q in range(4):
                    coff = E0_OFF + P * (4 * g + q)
                    nc.tensor.matmul(
                        (tA, tB)[q // 2][:, q % 2, 128:256],
                        a_sb[32 * q:32 * q + 4, P * g:P * g + P],
                        b_sb[32 * q:32 * q + 4, coff:coff + 128],
                        start=True,
                        stop=True,
                        tile_position=(32 * q, 0),
                    )
                reduce_tile(tA[:, :, 0:256], use_dve=True)
                reduce_tile(tB[:, :, 0:256], use_dve=True)

            assert job == NJOBS, job
            nc.sync.dma_start(acc_dram.ap()[:, :], acc_sb[:, :])

    nc.compile()
    return nc


def _get_nc():
    if "nc" not in _cache:
        _cache["nc"] = _build()
    return _cache["nc"]


def _make_in_maps(p, t, f, u):
    import ml_dtypes

    A = np.ascontiguousarray(
        np.stack([f * u, f, f * p, f * t]).astype(ml_dtypes.bfloat16)
    )
    Bm = np.ascontiguousarray(
        np.stack([f, f * u, -f * t, -f * p]).astype(ml_dtypes.bfloat16)
    )
    Bh = Bm * np.asarray(0.5, dtype=ml_dtypes.bfloat16)  # exact halving

    in_maps = []
    for k in range(NCORE):
        # a_rows layout: row 32q+r = factor r of row-block 4g+q, cols 128g..+128
        a_rows = np.zeros((P, 2 * P), dtype=ml_dtypes.bfloat16)
        for g in range(2):
            for q in range(4):
                a = 4 * g + q
                rows = slice(1024 * k + P * a, 1024 * k + P * a + P)
                a_rows[32 * q:32 * q + 4, P * g:P * g + P] = A[:, rows]

        b_cols = np.empty((4, BCOLS), dtype=ml_dtypes.bfloat16)
        cols = (1024 * k + np.arange(BMAIN)) % B
        b_cols[:, 0:BMAIN] = Bm[:, cols]
        e0_cols = (1024 * k + np.arange(ABLK * P)) % B
        b_cols[:, E0_OFF:E0_OFF + ABLK * P] = Bh[:, e0_cols]
        e32_cols = (1024 * k + 4096 + np.arange(ABLK * P)) % B
        b_cols[:, E32_OFF:E32_OFF + ABLK * P] = Bh[:, e32_cols]
        in_maps.append(
            {"a_rows": a_rows, "b_cols": np.ascontiguousarray(b_cols)}
        )
    return in_maps, A, Bm


def kernel(pred, gt, gt_fracTime, gt_ifMOF):
    from concourse import bass_utils

    pred = np.asarray(pred)
    gt = np.asarray(gt)
    ift = int(np.asarray(gt_fracTime))
    imf = int(np.asarray(gt_ifMOF))

    p = pred.astype(np.float32)
    t = gt[:, ift].astype(np.float32)
    f = (gt[:, imf] == 1).astype(np.float32)
    u = (p * t).astype(np.float32)

    in_maps, A, Bm = _make_in_maps(p, t, f, u)
    nc = _get_nc()
    res = bass_utils.run_bass_kernel_spmd(nc, in_maps, core_ids=list(range(NCORE)))

    # T = sum_{i<j} ff |M| (all device accumulator columns are weight 1)
    T = 0.0
    for r in res.results:
        T += r["acc"].astype(np.float64).sum()

    # host closed form in fp64 over the same bf16 values the device used:
    # sum_{i<j} M = (sum_{i,j} M - sum_diag M) / 2
    A64 = A.astype(np.float64)
    B64 = Bm.astype(np.float64)
    S_all = (A64.sum(axis=1) * B64.sum(axis=1)).sum()
    D_diag = (A64 * B64).sum()
    S_half = (S_all - D_diag) / 2.0

    f64 = f.astype(np.float64)
    S_f = f64.sum()
    n_pairs = (S_f * S_f - S_f) / 2.0

    loss = 0.5 * (S_half + T) / 100.0 / n_pairs
    return np.asarray(np.float32(loss))



# revision 2
# speedup vs baseline: 1.5050x; 1.5050x over previous
"""Trainium2 Bass kernel for the pairwise concordance-index loss.

reference:
    loss = sum_{i<j, f_i=f_j=1} relu((p_i-p_j)(t_i-t_j)) / 100 / n_pairs

Math:
  Compact to the n1 flagged entries (f=1), pad with zero rows to NPAD.
  M[i,j] = (p_i-p_j)(t_i-t_j) = A^T B, rank 4:
      A = [u, 1, p, t],  B = [1, u, -t, -p],  u = p*t   (bf16)
  sum relu(M) = 0.5*(sum M + sum |M|); sum M over i<j has an O(n) closed
  form done on the host in fp64 over the same bf16 factors; sum |M| is the
  O(n^2) part.

Work split (NB = NPAD/128 row/col blocks, NB=48 for this input):
  Device: for each row-block A, the cyclic column-blocks at offsets
  e = 1..NB/2-1; |M| is symmetric so each unordered block pair is computed
  exactly once.  Host (exact fp64 over the bf16 factors): e=0 (within-block
  pairs) and e=NB/2 block pairs.

Device (8 cores, identical program, data-sharded): core k owns row-blocks
  6k..6k+5 as 3 pairs.  Per pair (blocks bA,bB): 23*128=2944 columns each,
  generated by K=4 bf16 matmuls packed 4-way into disjoint 32-row PE groups
  (tile_position) -> 4-bank PSUM supertiles [128,4,512].  Each supertile is
  consumed by one abs-row-sum job on the DVE (tensor_reduce XY,
  apply_absolute_value) or the ScalarE (activation Abs + accum_out),
  assigned greedily to balance predicted engine time.
"""

import numpy as np

P = 128
NCORE = 8

_cache = {}


def _plan(nb):
    """Static per-core schedule: NB total blocks, nb_core = nb//NCORE (even).

    Returns list of pairs; per pair the quads and reduce jobs.
    Local block j (0..nb_core-1) main columns at b_sb offset 128*j..+2944.
    """
    nb_core = nb // NCORE
    assert nb_core % 2 == 0
    ncols = (nb // 2 - 1) * P          # 2944 device cols per block
    nchunk = ncols // 512              # full 512 chunks (5)
    rem = ncols - nchunk * 512         # 384 remainder
    pairs = []
    for pr in range(nb_core // 2):
        jA, jB = 2 * pr, 2 * pr + 1
        # chunks: (local block, col offset, N)
        A = [(jA, P * jA + 512 * c, 512) for c in range(nchunk)]
        B = [(jB, P * jB + 512 * c, 512) for c in range(nchunk)]
        A.append((jA, P * jA + 512 * nchunk, rem))
        B.append((jB, P * jB + 512 * nchunk, rem))
        # quads: 4 concurrent matmuls (one per PE row-group) -> 4 banks
        quads = [
            [A[0], A[1], B[0], B[1]],          # S1: FD 2048
            [A[2], A[3], B[2], B[3]],          # S2: FD 2048
            [A[4], B[4], A[5], B[5]],          # S3: 512,512,384,384
        ]
        pairs.append(quads)
    return pairs


def _build(nb):
    """Build + compile the Bass module (once per process)."""
    import concourse.bacc as bacc
    import concourse.tile as tile
    import concourse.mybir as mybir

    f32 = mybir.dt.float32
    bf16 = mybir.dt.bfloat16
    nb_core = nb // NCORE
    ncols = (nb // 2 - 1) * P
    awidth = P * (nb_core // 2)        # a_rows cols: one 128-col slab per pair
    bwidth = P * (nb_core - 1) + ncols # b_cols width (3584 for nb=48)
    pairs = _plan(nb)
    njobs = sum(4 for _ in pairs)      # S1, S2, S3a, S3b per pair

    nc = bacc.Bacc("TRN2", target_bir_lowering=False, debug=False,
                   num_devices=NCORE)
    a_dram = nc.dram_tensor("a_rows", [P, awidth], bf16, kind="ExternalInput")
    b_dram = nc.dram_tensor("b_cols", [4, bwidth], bf16, kind="ExternalInput")
    acc_dram = nc.dram_tensor("acc", [P, njobs], f32, kind="ExternalOutput")

    # predicted job cost (ns) per engine, used for greedy balancing
    def dve_cost(fd):
        return (120 + fd) / 0.96 + 36

    def act_cost(fd):
        return (fd + 540) / 1.2 + 33

    with tile.TileContext(nc) as tc:
        with (
            tc.tile_pool(name="inp", bufs=1) as inp_pool,
            tc.tile_pool(name="accp", bufs=1) as acc_pool,
            tc.tile_pool(name="ps", bufs=2, space="PSUM") as ps,
        ):
            a_sb = inp_pool.tile([P, awidth], bf16)
            b_sb = inp_pool.tile([P, bwidth], bf16)
            acc_sb = acc_pool.tile([P, njobs], f32)

            nc.sync.dma_start(a_sb[:, :], a_dram.ap()[:, :])
            # replicate the 4 B-factor rows into all four 32-row PE groups;
            # two queues (SP + ACT HWDGE) to halve the serial trigger cost
            nc.sync.dma_start(b_sb[0:4, :], b_dram.ap()[:, :])
            nc.scalar.dma_start(b_sb[32:36, :], b_dram.ap()[:, :])
            nc.sync.dma_start(b_sb[64:68, :], b_dram.ap()[:, :])
            nc.scalar.dma_start(b_sb[96:100, :], b_dram.ap()[:, :])

            t_dve = 0.0
            t_act = 0.0
            job = 0

            def reduce_job(ap, fd):
                nonlocal t_dve, t_act, job
                if t_dve + dve_cost(fd) <= t_act + act_cost(fd):
                    t_dve += dve_cost(fd)
                    nc.vector.tensor_reduce(
                        acc_sb[:, job:job + 1], ap,
                        axis=mybir.AxisListType.XY, op=mybir.AluOpType.add,
                        apply_absolute_value=True,
                    )
                else:
                    t_act += act_cost(fd)
                    nc.scalar.activation(
                        ap, ap, mybir.ActivationFunctionType.Abs,
                        accum_out=acc_sb[:, job:job + 1],
                    )
                job += 1

            for pr, quads in enumerate(pairs):
                for qi, quad in enumerate(quads):
                    st = ps.tile([P, 4, 512], f32, tag="st", name=f"st{pr}_{qi}")
                    for g, (j, off, n) in enumerate(quad):
                        # weights for local block j live in a_rows at pair
                        # slab 128*(j//2); groups 0,1 hold block 2p, groups
                        # 2,3 hold block 2p+1 (same layout host-side)
                        nc.tensor.matmul(
                            st[:, g, 0:n],
                            a_sb[32 * g:32 * g + 4, P * pr:P * pr + P],
                            b_sb[32 * g:32 * g + 4, off:off + n],
                            start=True, stop=True,
                            tile_position=(32 * g, 0),
                        )
                    if qi < 2:
                        reduce_job(st[:, :, :], 2048)
                    else:
                        reduce_job(st[:, 0:2, :], 1024)
                        reduce_job(st[:, 2:4, 0:384], 768)

            assert job == njobs, job
            nc.sync.dma_start(acc_dram.ap()[:, :], acc_sb[:, :])

    nc.compile()
    return nc


def _get_nc(nb):
    key = ("nc", nb)
    if key not in _cache:
        _cache[key] = _build(nb)
    return _cache[key]


def _factors(p, t):
    """bf16 rank-4 factors for compacted (all-flagged) entries p, t."""
    import ml_dtypes

    u = (p * t).astype(np.float32)
    one = np.ones_like(p)
    A = np.stack([u, one, p, t]).astype(ml_dtypes.bfloat16)
    B = np.stack([one, u, -t, -p]).astype(ml_dtypes.bfloat16)
    return A, B


def _make_in_maps(A, Bm, nb):
    """Per-core DRAM images from padded bf16 factor matrices [4, NPAD]."""
    npad = nb * P
    nb_core = nb // NCORE
    ncols = (nb // 2 - 1) * P
    awidth = P * (nb_core // 2)
    bwidth = P * (nb_core - 1) + ncols

    in_maps = []
    for k in range(NCORE):
        a_rows = np.zeros((P, awidth), dtype=A.dtype)
        for pr in range(nb_core // 2):
            bA = nb_core * k + 2 * pr
            bB = bA + 1
            for g in range(4):
                blk = bA if g < 2 else bB
                a_rows[32 * g:32 * g + 4, P * pr:P * pr + P] = \
                    A[:, P * blk:P * blk + P]
        cols = (P * (nb_core * k + 1) + np.arange(bwidth)) % npad
        b_cols = np.ascontiguousarray(Bm[:, cols])
        in_maps.append({"a_rows": a_rows, "b_cols": b_cols})
    return in_maps


def kernel(pred, gt, gt_fracTime, gt_ifMOF):
    from concourse import bass_utils

    pred = np.asarray(pred)
    gt = np.asarray(gt)
    ift = int(np.asarray(gt_fracTime))
    imf = int(np.asarray(gt_ifMOF))

    p_full = pred.astype(np.float32)
    t_full = gt[:, ift].astype(np.float32)
    f_full = gt[:, imf] == 1

    idx = np.nonzero(f_full)[0]
    n1 = len(idx)
    p = p_full[idx]
    t = t_full[idx]

    # pad so blocks split into 8 cores * (even block count)
    gran = NCORE * 2 * P
    npad = max(gran * 2, ((n1 + gran - 1) // gran) * gran)
    nb = npad // P

    A, Bm = _factors(p, t)
    Ap = np.zeros((4, npad), dtype=A.dtype)
    Bp = np.zeros((4, npad), dtype=Bm.dtype)
    Ap[:, :n1] = A
    Bp[:, :n1] = Bm

    in_maps = _make_in_maps(Ap, Bp, nb)
    nc = _get_nc(nb)
    res = bass_utils.run_bass_kernel_spmd(nc, in_maps,
                                          core_ids=list(range(NCORE)))

    # device part: sum |M| over block pairs at cyclic offsets 1..NB/2-1
    T = 0.0
    for r in res.results:
        T += r["acc"].astype(np.float64).sum()

    # host parts in fp64 over the same bf16 values the device used
    A64 = Ap.astype(np.float64).reshape(4, nb, P)
    B64 = Bp.astype(np.float64).reshape(4, nb, P)
    # e=0: within-block pairs i<j  (diagonal i=j excluded exactly)
    Md = np.einsum('kba,kbc->bac', A64, B64)
    for b in range(nb):
        np.fill_diagonal(Md[b], 0.0)
    T += 0.5 * np.abs(Md).sum()
    # e=NB/2 block pairs, each unordered pair once
    h = nb // 2
    Me = np.einsum('kba,kbc->bac', A64[:, :h], B64[:, h:])
    T += np.abs(Me).sum()

    # signed closed form: sum_{i<j} M = (sum_{ij} M - sum_ii M)/2
    Af = Ap.astype(np.float64)
    Bf = Bp.astype(np.float64)
    S_all = (Af.sum(axis=1) * Bf.sum(axis=1)).sum()
    D_diag = (Af * Bf).sum()
    S_half = (S_all - D_diag) / 2.0

    n1f = float(n1)
    n_pairs = (n1f * n1f - n1f) / 2.0
    loss = 0.5 * (S_half + T) / 100.0 / n_pairs
    return np.asarray(np.float32(loss))
